# revision 12
# baseline (speedup 1.0000x reference)
"""Trainium2 Bass kernel for nn_AttentionBlock_68624987455817.

Pre-LN causal self-attention block + MLP (B=8, L=1024, E=768, H=12, D=64).

Sharding: data-parallel over batch B=8 across the 8 NeuronCores (one batch
element per core, weights replicated, no collectives). Each core runs the
full block on its [1024, 768] slice.

Per-core dataflow (activations kept feature-major through the matmuls so no
transposes are needed inside attention):
  ph0   LN1 on token-major x tiles; transpose z1 -> z1T [E, L]
  ph1   qkT = wqk^T @ z1T           (feature-major q,k; q pre-scaled 1/sqrt(D))
  ph2   v   = z1 @ wv               (token-major, lhsT = z1T tiles)
  ph3   per head: S^T = k_h^T q_h -> exp -> P^T (masked);
        [O^T; sums] = [V_h | 1]^T @ P^T  (col-packed ones block), normalize
  ph4   x1 = O @ wproj + x          (token-major residual; x1 -> DRAM scratch)
  ph4.5 LN2 on x1 tiles; transpose z2 -> z2T
  ph5   hT = selu(wfc^T @ z2T)      (wfc pre-scaled by selu lambda)
  ph6   out = h @ wout + x1         (token-major, two column passes)

Matmuls run in float32r (full-rate fp32 on the PE, reduced multiply
precision); accumulation and everything else is fp32. Softmax skips the
max-subtraction (|S| is O(10) for LN'd inputs so exp cannot overflow in
fp32); causal masking zeroes P^T blocks above the diagonal.

LN scales are folded into the following weight matrices host-side; LN biases
and all linear biases fold into per-feature biases that are only materialized
on-chip when nonzero (they are all zero for this problem's inputs).
"""
import sys

sys.path.insert(0, "/opt/trn_rl_repo")

import numpy as np

import concourse.bass as bass
from concourse import bacc
import concourse.mybir as mybir
from concourse.tile import TileContext
from concourse import bass_utils
from concourse.masks import make_identity

F32 = mybir.dt.float32
F32R = mybir.dt.float32r
AF = mybir.ActivationFunctionType
OP = mybir.AluOpType
AX = mybir.AxisListType

P = 128
L = 1024
E = 768
H = 12
D = 64
EC = E // P          # 6 feature chunks
LT = L // P          # 8 token tiles
QC = L // 512        # 2 query chunks
KC2 = 4 * E // P     # 24 chunks of the MLP hidden dim
NCORES = 8

SELU_LAMBDA = 1.0507009873554805
SELU_ALPHA = 1.6732632423543772
SELU_LA = SELU_LAMBDA * SELU_ALPHA
LN_EPS = 1e-6

_last_results = None
_build_cache = {}


def _build(gates):
    nc = bacc.Bacc("TRN2", target_bir_lowering=False)

    x_d = nc.dram_tensor("x", [L, E], F32, kind="ExternalInput")
    wqk_d = nc.dram_tensor("wqk", [E, 2 * E], F32R, kind="ExternalInput")
    wv_d = nc.dram_tensor("wv", [E, E], F32R, kind="ExternalInput")
    wproj_d = nc.dram_tensor("wproj", [E, E], F32R, kind="ExternalInput")
    wfc_d = nc.dram_tensor("wfc", [E, 4 * E], F32R, kind="ExternalInput")
    wout_d = nc.dram_tensor("wout", [4 * E, E], F32R, kind="ExternalInput")
    out_d = nc.dram_tensor("out", [L, E], F32, kind="ExternalOutput")
    x1_d = nc.dram_tensor("x1_scratch", [L, E], F32, kind="Internal")

    bqk_d = bv_d = bproj_d = bfce_d = bfcl_d = bout_d = None
    if gates["bqk"]:
        bqk_d = nc.dram_tensor("bqk", [2 * E], F32, kind="ExternalInput")
    if gates["bv"]:
        bv_d = nc.dram_tensor("bv", [E], F32, kind="ExternalInput")
    if gates["bproj"]:
        bproj_d = nc.dram_tensor("bproj", [E], F32, kind="ExternalInput")
    if gates["bfc"]:
        bfce_d = nc.dram_tensor("bfce", [4 * E], F32, kind="ExternalInput")
        bfcl_d = nc.dram_tensor("bfcl", [4 * E], F32, kind="ExternalInput")
    if gates["bout"]:
        bout_d = nc.dram_tensor("bout", [E], F32, kind="ExternalInput")

    xv = x_d.rearrange("(t p) e -> p t e", p=P)            # [128, 8, 768]
    wqkv = wqk_d.rearrange("(c p) m -> p c m", p=P)        # [128, 6, 1536]
    wvv = wv_d.rearrange("(c p) m -> p c m", p=P)          # [128, 6, 768]
    wprojv = wproj_d.rearrange("(c p) m -> p c m", p=P)    # [128, 6, 768]
    wfcv = wfc_d.rearrange("(c p) m -> p c m", p=P)        # [128, 6, 3072]
    woutv = wout_d.rearrange("(c p) m -> p c m", p=P)      # [128, 24, 768]
    outv = out_d.rearrange("(t p) e -> p t e", p=P)
    x1v = x1_d.rearrange("(t p) e -> p t e", p=P)

    with TileContext(nc) as tc:
        with tc.tile_pool(name="persist", bufs=1) as pers:
            ident = pers.tile([P, P], F32)
            make_identity(nc, ident)
            # mask_tri[p, f] = 1.0 if f >= p else 0.0 (keep where k <= q).
            # Built in f32 (f32r memset/affine_select fail walrus codegen);
            # bitcast to f32r at the point of use.
            mask_f32 = pers.tile([P, P], F32)
            nc.gpsimd.memset(mask_f32[:], 0.0)
            nc.gpsimd.affine_select(
                out=mask_f32[:], in_=mask_f32[:],
                compare_op=OP.is_ge, fill=1.0, base=-1,
                pattern=[[-1, P]], channel_multiplier=1,
            )
            mask_tri = mask_f32[:].bitcast(F32R)
            ones_f32 = pers.tile([P, D], F32)
            nc.vector.memset(ones_f32[:], 1.0)
            ones64 = pers.tile([P, D], F32R)
            nc.vector.tensor_copy(ones64[:], ones_f32[:])
            eps_b = pers.tile([P, 1], F32)
            nc.vector.memset(eps_b[:], LN_EPS)
            lnla_b = pers.tile([P, 1], F32)
            nc.vector.memset(lnla_b[:], float(np.log(SELU_LA)))

            m1 = pers.tile([P, LT], F32)
            sq1 = pers.tile([P, LT], F32)
            r1 = pers.tile([P, LT], F32)
            tmp8 = pers.tile([P, LT], F32)
            m2 = pers.tile([P, LT], F32)
            sq2 = pers.tile([P, LT], F32)
            r2 = pers.tile([P, LT], F32)

            bqk_sb = bv_sb = bproj_sb = bfce_sb = bfcl_sb = bout_sb = None
            if gates["bqk"]:
                bqk_sb = pers.tile([P, 2 * EC], F32)
                nc.sync.dma_start(bqk_sb[:], bqk_d.rearrange("(c p) -> p c", p=P))
            if gates["bv"]:
                bv_sb = pers.tile([P, E], F32)
                nc.sync.dma_start(bv_sb[:], bv_d.to_broadcast((P, E)))
            if gates["bproj"]:
                bproj_sb = pers.tile([P, E], F32)
                nc.sync.dma_start(bproj_sb[:], bproj_d.to_broadcast((P, E)))
            if gates["bfc"]:
                bfce_sb = pers.tile([P, KC2], F32)
                nc.sync.dma_start(bfce_sb[:], bfce_d.rearrange("(c p) -> p c", p=P))
                bfcl_sb = pers.tile([P, KC2], F32)
                nc.sync.dma_start(bfcl_sb[:], bfcl_d.rearrange("(c p) -> p c", p=P))
            if gates["bout"]:
                bout_sb = pers.tile([P, E], F32)
                nc.sync.dma_start(bout_sb[:], bout_d.to_broadcast((P, E)))

            with tc.tile_pool(name="fm", bufs=1) as fmp:
                # ---------------- ph0: LN1 + transpose -> z1T ----------------
                z1T = fmp.tile([P, EC, L], F32R, tag="fm")
                with (
                    tc.tile_pool(name="ph0x", bufs=LT) as xp,
                    tc.tile_pool(name="ph0z", bufs=3) as zp,
                    tc.tile_pool(name="ph0s", bufs=2) as scr,
                    tc.tile_pool(name="ps0", bufs=4, space="PSUM") as ps0,
                ):
                    xtiles = []
                    for t in range(LT):
                        xt = xp.tile([P, E], F32, tag="x")
                        nc.sync.dma_start(xt[:], xv[:, t, :])
                        nc.vector.tensor_reduce(m1[:, t:t + 1], xt[:], AX.X, OP.add)
                        sqs = scr.tile([P, E], F32, tag="sq")
                        nc.scalar.activation(sqs[:], xt[:], AF.Square,
                                             accum_out=sq1[:, t:t + 1])
                        xtiles.append(xt)
                    nc.vector.tensor_scalar_mul(m1[:], m1[:], 1.0 / E)
                    nc.vector.tensor_scalar_mul(sq1[:], sq1[:], 1.0 / E)
                    nc.vector.tensor_tensor(tmp8[:], m1[:], m1[:], OP.mult)
                    nc.vector.tensor_tensor(sq1[:], sq1[:], tmp8[:], OP.subtract)
                    nc.scalar.activation(sq1[:], sq1[:], AF.Sqrt, bias=eps_b[:])
                    nc.vector.reciprocal(r1[:], sq1[:])
                    for t in range(LT):
                        zt = zp.tile([P, E], F32, tag="z")
                        nc.vector.tensor_scalar(
                            zt[:], xtiles[t][:], m1[:, t:t + 1], r1[:, t:t + 1],
                            OP.subtract, OP.mult,
                        )
                        for c in range(EC):
                            pt = ps0.tile([P, P], F32, tag="tr")
                            nc.tensor.transpose(pt[:], zt[:, c * P:(c + 1) * P], ident[:])
                            nc.any.tensor_copy(out=z1T[:, c, t * P:(t + 1) * P], in_=pt[:])

                # ---------------- ph1+2: qkT, v ----------------
                with (
                    tc.tile_pool(name="qkp", bufs=1) as qkpool,
                    tc.tile_pool(name="vp", bufs=1) as vpool,
                ):
                    qkT = qkpool.tile([P, 2 * EC, L], F32R)
                    v_sb = vpool.tile([P, LT, E], F32R)
                    with (
                        tc.tile_pool(name="wqks", bufs=3) as wqs,
                        tc.tile_pool(name="ps1", bufs=4, space="PSUM") as ps1,
                    ):
                        for oc in range(2 * EC):
                            wt = wqs.tile([P, EC, P], F32R, tag="wqk")
                            nc.sync.dma_start(wt[:], wqkv[:, :, oc * P:(oc + 1) * P])
                            psums = [ps1.tile([P, 512], F32, tag="mm", name=f"qkps{lc}")
                                     for lc in range(QC)]
                            for kc in range(EC):
                                for lc in range(QC):
                                    nc.tensor.matmul(
                                        psums[lc][:], wt[:, kc, :],
                                        z1T[:, kc, lc * 512:(lc + 1) * 512],
                                        start=(kc == 0), stop=(kc == EC - 1),
                                    )
                            for lc in range(QC):
                                dst = qkT[:, oc, lc * 512:(lc + 1) * 512]
                                if gates["bqk"]:
                                    nc.scalar.activation(dst, psums[lc][:], AF.Identity,
                                                         bias=bqk_sb[:, oc:oc + 1])
                                else:
                                    nc.any.tensor_copy(out=dst, in_=psums[lc][:])

                    with (
                        tc.tile_pool(name="wvp", bufs=1) as wvp,
                        tc.tile_pool(name="ps2", bufs=4, space="PSUM") as ps2,
                    ):
                        wv_sb = wvp.tile([P, EC, E], F32R)
                        nc.sync.dma_start(wv_sb[:], wvv[:])
                        for t in range(LT):
                            for (c0, cw) in ((0, 512), (512, 256)):
                                pt = ps2.tile([P, 512], F32, tag="mm")
                                for kc in range(EC):
                                    nc.tensor.matmul(
                                        pt[:, :cw], z1T[:, kc, t * P:(t + 1) * P],
                                        wv_sb[:, kc, c0:c0 + cw],
                                        start=(kc == 0), stop=(kc == EC - 1),
                                    )
                                dst = v_sb[:, t, c0:c0 + cw]
                                if gates["bv"]:
                                    nc.vector.tensor_tensor(dst, pt[:, :cw],
                                                            bv_sb[:, c0:c0 + cw], OP.add)
                                else:
                                    nc.any.tensor_copy(out=dst, in_=pt[:, :cw])

                    # ---------------- ph3: attention ----------------
                    with (
                        tc.tile_pool(name="ptp", bufs=1) as ptp,
                        tc.tile_pool(name="recp", bufs=4) as recp,
                        tc.tile_pool(name="ps3s", bufs=2, space="PSUM") as ps3s,
                        tc.tile_pool(name="ps3v", bufs=2, space="PSUM") as ps3v,
                    ):
                        OT = fmp.tile([P, EC, L], F32R, tag="fm")
                        PT = [ptp.tile([P, LT, L], F32R, tag=f"pt{i}", name=f"pt{i}")
                              for i in range(2)]
                        # f32r memsets fail walrus codegen; write zeros via a
                        # converting copy from an f32 zero tile instead.
                        zsrc = recp.tile([P, (LT - 1) * P], F32, tag="zs")
                        nc.vector.memset(zsrc[:], 0.0)
                        for i in range(2):
                            for kt in range(1, LT):
                                nc.vector.tensor_copy(PT[i][:, kt, 0:kt * P],
                                                      zsrc[:, 0:kt * P])

                        for h in range(H):
                            par = h % 2
                            c = h // 2
                            off = par * D
                            rows = slice(off, off + D)
                            pt_buf = PT[h % 2]
                            for qc in range(QC):
                                for kt in range(4 * qc, 4 * (qc + 1)):
                                    s0 = kt * P
                                    pss = ps3s.tile([P, L], F32, tag="st")
                                    if s0 < 512:
                                        segs = [(s0, 512), (512, L)]
                                    else:
                                        segs = [(s0, L)]
                                    lhs = qkT[rows, EC + c, s0:s0 + P]
                                    for (a, b) in segs:
                                        nc.tensor.matmul(pss[:, a:b], lhs,
                                                         qkT[rows, c, a:b],
                                                         start=True, stop=True)
                                    nc.scalar.activation(pt_buf[:, kt, s0:L],
                                                         pss[:, s0:L], AF.Exp)
                                    nc.vector.tensor_tensor(
                                        pt_buf[:, kt, s0:s0 + P],
                                        pt_buf[:, kt, s0:s0 + P],
                                        mask_tri[:], OP.mult,
                                    )
                                # P@V for this query chunk (kt 0..4*(qc+1)-1).
                                # fp32r matmuls may only target psum partition
                                # base 0, so the ones (row-sum) matmul gets its
                                # own psum tile; cross-partition-base DVE ops
                                # put the result at the head's OT rows.
                                q0 = qc * 512
                                pso_v = ps3v.tile([P, 512], F32, tag="pv")
                                pso_s = ps3v.tile([P, 512], F32, tag="pvs")
                                kts = list(range(4 * (qc + 1)))
                                for j, kt in enumerate(kts):
                                    st = (j == 0)
                                    sp = (j == len(kts) - 1)
                                    a = max(kt * P, q0)
                                    vsl = v_sb[:, kt, h * D:(h + 1) * D]
                                    rhs = pt_buf[:, kt, a:q0 + 512]
                                    dst = slice(a - q0, 512)
                                    nc.tensor.matmul(pso_v[0:D, dst], vsl, rhs,
                                                     start=st, stop=sp)
                                    nc.tensor.matmul(pso_s[0:D, dst], ones64[:], rhs,
                                                     start=st, stop=sp)
                                o_rows = slice(off, off + D)
                                rec = recp.tile([P, 512], F32, tag="rc")
                                nc.vector.reciprocal(rec[o_rows, :], pso_s[0:D, :])
                                nc.vector.tensor_tensor(
                                    OT[o_rows, c, q0:q0 + 512],
                                    pso_v[0:D, :], rec[o_rows, :], OP.mult,
                                )

                # ---------------- ph4: proj + residual (+ LN2 stats) --------
                with (
                    tc.tile_pool(name="wpp", bufs=1) as wpp,
                    tc.tile_pool(name="xrp", bufs=3) as xrp,
                    tc.tile_pool(name="x1p", bufs=3) as x1p,
                    tc.tile_pool(name="scr4", bufs=2) as scr4,
                    tc.tile_pool(name="ps4", bufs=4, space="PSUM") as ps4,
                ):
                    wproj_sb = wpp.tile([P, EC, E], F32R)
                    nc.sync.dma_start(wproj_sb[:], wprojv[:])
                    for t in range(LT):
                        xt = xrp.tile([P, E], F32, tag="xr")
                        nc.sync.dma_start(xt[:], xv[:, t, :])
                        x1t = x1p.tile([P, E], F32, tag="x1")
                        for (c0, cw) in ((0, 512), (512, 256)):
                            pt = ps4.tile([P, 512], F32, tag="mm")
                            for kc in range(EC):
                                nc.tensor.matmul(
                                    pt[:, :cw], OT[:, kc, t * P:(t + 1) * P],
                                    wproj_sb[:, kc, c0:c0 + cw],
                                    start=(kc == 0), stop=(kc == EC - 1),
                                )
                            dst = x1t[:, c0:c0 + cw]
                            if gates["bproj"]:
                                nc.vector.tensor_tensor(dst, pt[:, :cw],
                                                        bproj_sb[:, c0:c0 + cw], OP.add)
                                nc.vector.tensor_tensor(dst, dst,
                                                        xt[:, c0:c0 + cw], OP.add)
                            else:
                                nc.vector.tensor_tensor(dst, pt[:, :cw],
                                                        xt[:, c0:c0 + cw], OP.add)
                        nc.sync.dma_start(x1v[:, t, :], x1t[:])
                        nc.vector.tensor_reduce(m2[:, t:t + 1], x1t[:], AX.X, OP.add)
                        sqs = scr4.tile([P, E], F32, tag="sq4")
                        nc.scalar.activation(sqs[:], x1t[:], AF.Square,
                                             accum_out=sq2[:, t:t + 1])
                    nc.vector.tensor_scalar_mul(m2[:], m2[:], 1.0 / E)
                    nc.vector.tensor_scalar_mul(sq2[:], sq2[:], 1.0 / E)
                    nc.vector.tensor_tensor(tmp8[:], m2[:], m2[:], OP.mult)
                    nc.vector.tensor_tensor(sq2[:], sq2[:], tmp8[:], OP.subtract)
                    nc.scalar.activation(sq2[:], sq2[:], AF.Sqrt, bias=eps_b[:])
                    nc.vector.reciprocal(r2[:], sq2[:])

                # ---------------- ph4.5: LN2 apply + transpose -> z2T -------
                z2T = fmp.tile([P, EC, L], F32R, tag="fm")
                with (
                    tc.tile_pool(name="x1r", bufs=3) as x1r,
                    tc.tile_pool(name="z2p", bufs=3) as z2p,
                    tc.tile_pool(name="ps45", bufs=4, space="PSUM") as ps45,
                ):
                    for t in range(LT):
                        x1t = x1r.tile([P, E], F32, tag="x1r")
                        nc.sync.dma_start(x1t[:], x1v[:, t, :])
                        z2t = z2p.tile([P, E], F32, tag="z2")
                        nc.vector.tensor_scalar(
                            z2t[:], x1t[:], m2[:, t:t + 1], r2[:, t:t + 1],
                            OP.subtract, OP.mult,
                        )
                        for c in range(EC):
                            pt = ps45.tile([P, P], F32, tag="tr")
                            nc.tensor.transpose(pt[:], z2t[:, c * P:(c + 1) * P], ident[:])
                            nc.any.tensor_copy(out=z2T[:, c, t * P:(t + 1) * P], in_=pt[:])

                # ---------------- ph5: fc + selu -> hT ----------------------
                with tc.tile_pool(name="htp", bufs=1) as htp:
                    hT = htp.tile([P, KC2, L], F32R)
                    with (
                        tc.tile_pool(name="wfcs", bufs=3) as wfs,
                        tc.tile_pool(name="selu", bufs=4) as slp,
                        tc.tile_pool(name="ps5", bufs=4, space="PSUM") as ps5,
                    ):
                        ln_la = float(np.log(SELU_LA))
                        for oc in range(KC2):
                            wt = wfs.tile([P, EC, P], F32R, tag="wfc")
                            nc.sync.dma_start(wt[:], wfcv[:, :, oc * P:(oc + 1) * P])
                            for lc in range(QC):
                                pt = ps5.tile([P, 512], F32, tag="mm")
                                for kc in range(EC):
                                    nc.tensor.matmul(
                                        pt[:], wt[:, kc, :],
                                        z2T[:, kc, lc * 512:(lc + 1) * 512],
                                        start=(kc == 0), stop=(kc == EC - 1),
                                    )
                                pe_t = slp.tile([P, 512], F32, tag="pe")
                                bias = (bfce_sb[:, oc:oc + 1] if gates["bfc"] else lnla_b[:])
                                nc.scalar.activation(pe_t[:], pt[:], AF.Exp,
                                                     bias=bias, scale=1.0 / SELU_LAMBDA)
                                a_t = slp.tile([P, 512], F32, tag="at")
                                nc.vector.tensor_scalar(a_t[:], pe_t[:], SELU_LA, SELU_LA,
                                                        OP.min, OP.subtract)
                                dst = hT[:, oc, lc * 512:(lc + 1) * 512]
                                if gates["bfc"]:
                                    rl = slp.tile([P, 512], F32, tag="rl")
                                    nc.vector.tensor_scalar(rl[:], pt[:],
                                                            bfcl_sb[:, oc:oc + 1], 0.0,
                                                            OP.add, OP.max)
                                    nc.vector.tensor_tensor(dst, rl[:], a_t[:], OP.add)
                                else:
                                    nc.vector.scalar_tensor_tensor(
                                        dst, pt[:], 0.0, a_t[:], OP.max, OP.add)

                    # ---------------- ph6: out + residual, 2 column passes ---
                    for (c0, cw) in ((0, 512), (512, 256)):
                        with (
                            tc.tile_pool(name=f"wos{c0}", bufs=1) as wop,
                            tc.tile_pool(name=f"x1s{c0}", bufs=3) as x1s,
                            tc.tile_pool(name=f"os{c0}", bufs=3) as osp,
                            tc.tile_pool(name=f"ps6{c0}", bufs=4, space="PSUM") as ps6,
                        ):
                            wo = wop.tile([P, KC2, cw], F32R, tag=f"wo{c0}")
                            nc.sync.dma_start(wo[:], woutv[:, :, c0:c0 + cw])
                            for t in range(LT):
                                pt = ps6.tile([P, 512], F32, tag="mm")
                                for kc in range(KC2):
                                    nc.tensor.matmul(
                                        pt[:, :cw], hT[:, kc, t * P:(t + 1) * P],
                                        wo[:, kc, :],
                                        start=(kc == 0), stop=(kc == KC2 - 1),
                                    )
                                x1t = x1s.tile([P, 512], F32, tag="x1c")
                                nc.sync.dma_start(x1t[:, :cw], x1v[:, t, c0:c0 + cw])
                                ot = osp.tile([P, 512], F32, tag="ot")
                                if gates["bout"]:
                                    nc.vector.tensor_tensor(ot[:, :cw], pt[:, :cw],
                                                            bout_sb[:, c0:c0 + cw], OP.add)
                                    nc.vector.tensor_tensor(ot[:, :cw], ot[:, :cw],
                                                            x1t[:, :cw], OP.add)
                                else:
                                    nc.vector.tensor_tensor(ot[:, :cw], pt[:, :cw],
                                                            x1t[:, :cw], OP.add)
                                nc.sync.dma_start(outv[:, t, c0:c0 + cw], ot[:, :cw])

    nc.finalize()
    return nc


def kernel(**inputs):
    global _last_results

    def arr(name):
        return np.ascontiguousarray(np.asarray(inputs[name], dtype=np.float32))

    x = arr("x")                       # [8, 1024, 768]
    g1 = arr("ln1_scale")
    b1 = arr("ln1_bias")
    w_qkv = arr("w_qkv")               # [768, 2304]
    b_qkv = arr("b_qkv")
    w_proj = arr("w_proj")
    b_proj = arr("b_proj")
    g2 = arr("ln2_scale")
    b2 = arr("ln2_bias")
    w_fc = arr("w_fc")
    b_fc = arr("b_fc")
    w_out = arr("w_out")
    b_out = arr("b_out")

    qscale = np.float32(1.0 / np.sqrt(D))

    w3 = w_qkv.reshape(E, H, 3, D)
    qw = (w3[:, :, 0, :].reshape(E, E) * qscale)
    kw = w3[:, :, 1, :].reshape(E, E)
    vw = w3[:, :, 2, :].reshape(E, E)
    wqk = np.ascontiguousarray(
        np.concatenate([qw, kw], axis=1) * g1[:, None]).astype(np.float32)
    wv = np.ascontiguousarray(vw * g1[:, None]).astype(np.float32)

    bq3 = (b1 @ w_qkv + b_qkv).reshape(H, 3, D)
    bqk = np.concatenate(
        [bq3[:, 0, :].reshape(E) * qscale, bq3[:, 1, :].reshape(E)]).astype(np.float32)
    bv = np.ascontiguousarray(bq3[:, 2, :].reshape(E)).astype(np.float32)

    wfc_p = np.ascontiguousarray(
        w_fc * g2[:, None] * np.float32(SELU_LAMBDA)).astype(np.float32)
    bfc_eff = (b2 @ w_fc + b_fc).astype(np.float32)
    bfce = (bfc_eff + np.float32(np.log(SELU_LA))).astype(np.float32)
    bfcl = (bfc_eff * np.float32(SELU_LAMBDA)).astype(np.float32)

    gates = {
        "bqk": bool(np.any(bqk != 0)),
        "bv": bool(np.any(bv != 0)),
        "bproj": bool(np.any(b_proj != 0)),
        "bfc": bool(np.any(bfc_eff != 0)),
        "bout": bool(np.any(b_out != 0)),
    }

    key = tuple(sorted(gates.items()))
    if key not in _build_cache:
        _build_cache[key] = _build(gates)
    nc = _build_cache[key]

    base = {
        "wqk": wqk, "wv": wv,
        "wproj": np.ascontiguousarray(w_proj),
        "wfc": wfc_p,
        "wout": np.ascontiguousarray(w_out),
    }
    if gates["bqk"]:
        base["bqk"] = bqk
    if gates["bv"]:
        base["bv"] = bv
    if gates["bproj"]:
        base["bproj"] = np.ascontiguousarray(b_proj)
    if gates["bfc"]:
        base["bfce"] = bfce
        base["bfcl"] = bfcl
    if gates["bout"]:
        base["bout"] = np.ascontiguousarray(b_out)

    in_maps = [dict(base, x=np.ascontiguousarray(x[c])) for c in range(NCORES)]
    res = bass_utils.run_bass_kernel_spmd(nc, in_maps, core_ids=list(range(NCORES)))
    _last_results = res
    out = np.stack([res.results[c]["out"] for c in range(NCORES)], axis=0)
    return out.astype(np.float32)


# revision 18
# speedup vs baseline: 1.3571x; 1.3571x over previous
"""Trainium2 Bass kernel for nn_AttentionBlock_68624987455817.

Pre-LN causal self-attention block + MLP (B=8, L=1024, E=768, H=12, D=64).

Sharding: data-parallel over batch B=8 across the 8 NeuronCores (one batch
element per core, weights replicated, no collectives). Each core runs the
full block on its [1024, 768] slice.

Per-core dataflow (activations kept feature-major through the matmuls so no
transposes are needed inside attention):
  ph0   LN1 on token-major x tiles; transpose z1 -> z1T [E, L]
  ph2   v   = z1 @ wv               (token-major, lhsT = z1T tiles; an extra
        ones column per head makes the P@V matmul emit softmax row-sums)
  ph3   per head pair: qk chunks (q pre-scaled 1/sqrt(D)), then
        S^T = k_h^T q_h -> exp -> P^T (masked); [O^T; sums] = Vaug^T P^T;
        normalize via fast reciprocal + gpsimd partition broadcast.
        Interleaving qk matmuls with the ACT-heavy softmax keeps the PE
        dense so the HAM clock gate stays at full rate.
  ph4   x1 = O @ wproj + x          (token-major residual; x1 -> DRAM scratch)
  ph4.5 LN2 on x1 tiles; transpose z2 -> z2T
  ph5   hT = selu(wfc^T @ z2T)      (wfc pre-scaled by selu lambda)
  ph6   out = h @ wout + x1         (token-major, two column passes; wout
        pass-A prefetched during ph5)

Matmul operand dtype is selectable (KERNEL_MM_DT env): "f32r" (float32r,
~2 cyc/row, rel err ~2e-4) or "bf16" (1 cyc/row, rel err ~4e-3).
Accumulation is always fp32; LN stats, residuals and the output are fp32.
Softmax skips the max-subtraction (|S| <= ~8 for LN'd inputs so exp cannot
overflow in fp32); causal masking zeroes P^T blocks above the diagonal.

LN scales fold into the following weight matrices host-side; LN biases and
all linear biases fold into per-feature biases that are only materialized
on-chip when nonzero (all zero for this problem's inputs).
"""
import os
import sys

sys.path.insert(0, "/opt/trn_rl_repo")

import numpy as np
import ml_dtypes

import concourse.bass as bass
from concourse import bacc
import concourse.mybir as mybir
from concourse.tile import TileContext
from concourse import bass_utils
from concourse.masks import make_identity

F32 = mybir.dt.float32
F32R = mybir.dt.float32r
BF16 = mybir.dt.bfloat16
AF = mybir.ActivationFunctionType
OP = mybir.AluOpType
AX = mybir.AxisListType

P = 128
L = 1024
E = 768
H = 12
D = 64
DA = D + 1           # V columns + ones column (row-sum trick)
EC = E // P          # 6 feature chunks
LT = L // P          # 8 token tiles
QC = L // 512        # 2 query chunks
KC2 = 4 * E // P     # 24 chunks of the MLP hidden dim
NCORES = 8

SELU_LAMBDA = 1.0507009873554805
SELU_ALPHA = 1.6732632423543772
SELU_LA = SELU_LAMBDA * SELU_ALPHA
LN_EPS = 1e-6

_last_results = None
_build_cache = {}


def _build(gates, mm_dt_name):
    MDT = {"f32r": F32R, "bf16": BF16}[mm_dt_name]
    use_dma_transpose = (MDT == BF16)

    nc = bacc.Bacc("TRN2", target_bir_lowering=False)

    x_d = nc.dram_tensor("x", [L, E], F32, kind="ExternalInput")
    wqk_d = nc.dram_tensor("wqk", [E, 2 * E], MDT, kind="ExternalInput")
    wv_d = nc.dram_tensor("wv", [E, E], MDT, kind="ExternalInput")
    wproj_d = nc.dram_tensor("wproj", [E, E], MDT, kind="ExternalInput")
    wfc_d = nc.dram_tensor("wfc", [E, 4 * E], MDT, kind="ExternalInput")
    wout_d = nc.dram_tensor("wout", [4 * E, E], MDT, kind="ExternalInput")
    out_d = nc.dram_tensor("out", [L, E], F32, kind="ExternalOutput")
    x1_d = nc.dram_tensor("x1_scratch", [L, E], F32, kind="Internal")

    bqk_d = bv_d = bproj_d = bfce_d = bfcl_d = bout_d = None
    if gates["bqk"]:
        bqk_d = nc.dram_tensor("bqk", [2 * E], F32, kind="ExternalInput")
    if gates["bv"]:
        bv_d = nc.dram_tensor("bv", [E], F32, kind="ExternalInput")
    if gates["bproj"]:
        bproj_d = nc.dram_tensor("bproj", [E], F32, kind="ExternalInput")
    if gates["bfc"]:
        bfce_d = nc.dram_tensor("bfce", [4 * E], F32, kind="ExternalInput")
        bfcl_d = nc.dram_tensor("bfcl", [4 * E], F32, kind="ExternalInput")
    if gates["bout"]:
        bout_d = nc.dram_tensor("bout", [E], F32, kind="ExternalInput")

    xv = x_d.rearrange("(t p) e -> p t e", p=P)            # [128, 8, 768]
    wqkv = wqk_d.rearrange("(c p) m -> p c m", p=P)        # [128, 6, 1536]
    wvv = wv_d.rearrange("(c p) m -> p c m", p=P)          # [128, 6, 768]
    wprojv = wproj_d.rearrange("(c p) m -> p c m", p=P)    # [128, 6, 768]
    wfcv = wfc_d.rearrange("(c p) m -> p c m", p=P)        # [128, 6, 3072]
    woutv = wout_d.rearrange("(c p) m -> p c m", p=P)      # [128, 24, 768]
    outv = out_d.rearrange("(t p) e -> p t e", p=P)
    x1v = x1_d.rearrange("(t p) e -> p t e", p=P)

    with TileContext(nc) as tc:
        with tc.tile_pool(name="persist", bufs=1) as pers:
            # mask_tri[p, f] = 1.0 if f >= p else 0.0 (keep where k <= q).
            # Built in f32 (f32r memset/affine_select fail walrus codegen).
            mask_f32 = pers.tile([P, P], F32)
            nc.gpsimd.memset(mask_f32[:], 0.0)
            nc.gpsimd.affine_select(
                out=mask_f32[:], in_=mask_f32[:],
                compare_op=OP.is_ge, fill=1.0, base=-1,
                pattern=[[-1, P]], channel_multiplier=1,
            )
            if MDT == F32R:
                mask_tri = mask_f32[:].bitcast(F32R)
            else:
                mask_b = pers.tile([P, P], BF16)
                nc.vector.tensor_copy(mask_b[:], mask_f32[:])
                mask_tri = mask_b[:]
            ones_f32 = pers.tile([P, LT * H], F32)
            nc.vector.memset(ones_f32[:], 1.0)
            eps_b = pers.tile([P, 1], F32)
            nc.vector.memset(eps_b[:], LN_EPS)
            lnla_b = pers.tile([P, 1], F32)
            nc.vector.memset(lnla_b[:], float(np.log(SELU_LA)))

            m1 = pers.tile([P, LT], F32)
            sq1 = pers.tile([P, LT], F32)
            r1 = pers.tile([P, LT], F32)
            tmp8 = pers.tile([P, LT], F32)
            m2 = pers.tile([P, LT], F32)
            sq2 = pers.tile([P, LT], F32)
            r2 = pers.tile([P, LT], F32)

            bqk_sb = bv_sb = bproj_sb = bfce_sb = bfcl_sb = bout_sb = None
            if gates["bqk"]:
                bqk_sb = pers.tile([P, 2 * EC], F32)
                nc.sync.dma_start(bqk_sb[:], bqk_d.rearrange("(c p) -> p c", p=P))
            if gates["bv"]:
                bv_sb = pers.tile([P, E], F32)
                nc.sync.dma_start(bv_sb[:], bv_d.to_broadcast((P, E)))
            if gates["bproj"]:
                bproj_sb = pers.tile([P, E], F32)
                nc.sync.dma_start(bproj_sb[:], bproj_d.to_broadcast((P, E)))
            if gates["bfc"]:
                bfce_sb = pers.tile([P, KC2], F32)
                nc.sync.dma_start(bfce_sb[:], bfce_d.rearrange("(c p) -> p c", p=P))
                bfcl_sb = pers.tile([P, KC2], F32)
                nc.sync.dma_start(bfcl_sb[:], bfcl_d.rearrange("(c p) -> p c", p=P))
            if gates["bout"]:
                bout_sb = pers.tile([P, E], F32)
                nc.sync.dma_start(bout_sb[:], bout_d.to_broadcast((P, E)))

            if not use_dma_transpose:
                ident = pers.tile([P, P], F32)
                make_identity(nc, ident)
                ident_r = pers.tile([P, P], F32R)
                nc.vector.tensor_copy(ident_r[:], ident[:])

            def transpose_into(dstT, src_tile, t, pspool):
                """dstT[:, c, t*P:(t+1)*P] = src_tile[:, c*P:(c+1)*P].T for all c."""
                if use_dma_transpose:
                    nc.sync.dma_start_transpose(dstT[:, :, t * P:(t + 1) * P],
                                                src_tile[:])
                else:
                    for c in range(EC):
                        pt = pspool.tile([P, P], F32R, tag="tr")
                        nc.tensor.transpose(pt[:], src_tile[:, c * P:(c + 1) * P],
                                            ident_r[:])
                        nc.any.tensor_copy(out=dstT[:, c, t * P:(t + 1) * P],
                                           in_=pt[:])

            with tc.tile_pool(name="fm", bufs=1) as fmp:
                # ---------------- ph0: LN1 + transpose -> z1T ----------------
                z1T = fmp.tile([P, EC, L], MDT, tag="fm")
                with (
                    tc.tile_pool(name="ph0x", bufs=LT) as xp,
                    tc.tile_pool(name="ph0z", bufs=3) as zp,
                    tc.tile_pool(name="ph0s", bufs=2) as scr,
                    tc.tile_pool(name="ps0", bufs=4, space="PSUM") as ps0,
                ):
                    xtiles = []
                    for t in range(LT):
                        xt = xp.tile([P, E], F32, tag="x")
                        nc.sync.dma_start(xt[:], xv[:, t, :])
                        nc.vector.tensor_reduce(m1[:, t:t + 1], xt[:], AX.X, OP.add)
                        sqs = scr.tile([P, E], F32, tag="sq")
                        nc.scalar.activation(sqs[:], xt[:], AF.Square,
                                             accum_out=sq1[:, t:t + 1])
                        xtiles.append(xt)
                    nc.vector.tensor_scalar_mul(m1[:], m1[:], 1.0 / E)
                    nc.vector.tensor_scalar_mul(sq1[:], sq1[:], 1.0 / E)
                    nc.vector.tensor_tensor(tmp8[:], m1[:], m1[:], OP.mult)
                    nc.vector.tensor_tensor(sq1[:], sq1[:], tmp8[:], OP.subtract)
                    nc.scalar.activation(sq1[:], sq1[:], AF.Sqrt, bias=eps_b[:])
                    nc.vector.reciprocal(r1[:], sq1[:])
                    for t in range(LT):
                        zt = zp.tile([P, E], MDT, tag="z")
                        nc.vector.tensor_scalar(
                            zt[:], xtiles[t][:], m1[:, t:t + 1], r1[:, t:t + 1],
                            OP.subtract, OP.mult,
                        )
                        transpose_into(z1T, zt, t, ps0)

                # ------- ph2+3 fused: v, then per-head-pair qk + attention ---
                with (
                    tc.tile_pool(name="otp", bufs=1) as otp,
                    tc.tile_pool(name="vp", bufs=1) as vpool,
                ):
                    OT = otp.tile([P, EC, L], MDT)
                    # v with a ones column per head: lhsT [128, 65] per
                    # (kt, head) -> P@V also emits softmax row-sums at psum
                    # row 64.
                    v_aug = vpool.tile([P, LT, H, DA], MDT)
                    nc.vector.tensor_copy(
                        v_aug[:, :, :, D:DA],
                        ones_f32[:].rearrange("p (t h o) -> p t h o", h=H, o=1))
                    with (
                        tc.tile_pool(name="wvp", bufs=1) as wvp,
                        tc.tile_pool(name="ps2", bufs=4, space="PSUM") as ps2,
                    ):
                        wv_sb = wvp.tile([P, EC, E], MDT)
                        nc.sync.dma_start(wv_sb[:], wvv[:])
                        for t in range(LT):
                            for (c0, cw) in ((0, 512), (512, 256)):
                                pt = ps2.tile([P, 512], F32, tag="mm")
                                for kc in range(EC):
                                    nc.tensor.matmul(
                                        pt[:, :cw], z1T[:, kc, t * P:(t + 1) * P],
                                        wv_sb[:, kc, c0:c0 + cw],
                                        start=(kc == 0), stop=(kc == EC - 1),
                                    )
                                # scatter the 64-wide head slices into v_aug
                                h0 = c0 // D
                                nh = cw // D
                                dst = v_aug[:, t, h0:h0 + nh, 0:D]
                                if gates["bv"]:
                                    nc.vector.tensor_tensor(
                                        dst,
                                        pt[:, :cw].rearrange("p (h d) -> p h d", d=D),
                                        bv_sb[:, c0:c0 + cw].rearrange(
                                            "p (h d) -> p h d", d=D),
                                        OP.add)
                                else:
                                    nc.any.tensor_copy(
                                        out=dst,
                                        in_=pt[:, :cw].rearrange(
                                            "p (h d) -> p h d", d=D))

                    with (
                        tc.tile_pool(name="qkpp", bufs=2) as qkpp,
                        tc.tile_pool(name="wqks", bufs=2) as wqs,
                        tc.tile_pool(name="ptp", bufs=1) as ptp,
                        tc.tile_pool(name="zsp", bufs=1) as zsp,
                        tc.tile_pool(name="recp", bufs=2) as recp,
                        tc.tile_pool(name="psqk", bufs=2, space="PSUM") as psqk,
                        tc.tile_pool(name="ps3s", bufs=2, space="PSUM") as ps3s,
                        tc.tile_pool(name="ps3v", bufs=2, space="PSUM") as ps3v,
                    ):
                        PT = [ptp.tile([P, LT, L], MDT, tag=f"pt{i}", name=f"pt{i}")
                              for i in range(2)]
                        # f32r memsets fail walrus codegen; write zeros via a
                        # converting copy from an f32 zero tile instead.
                        zsrc = zsp.tile([P, (LT - 1) * P], F32, tag="zs")
                        nc.vector.memset(zsrc[:], 0.0)
                        for i in range(2):
                            for kt in range(1, LT):
                                nc.vector.tensor_copy(PT[i][:, kt, 0:kt * P],
                                                      zsrc[:, 0:kt * P])

                        for c in range(EC):  # head pair (2c, 2c+1)
                            # qk matmuls for this pair: oc=c (q), oc=EC+c (k)
                            qk_pair = qkpp.tile([P, 2, L], MDT, tag="qkpair")
                            for i, oc in enumerate((c, EC + c)):
                                wt = wqs.tile([P, EC, P], MDT, tag="wqk")
                                nc.sync.dma_start(wt[:],
                                                  wqkv[:, :, oc * P:(oc + 1) * P])
                                psums = [psqk.tile([P, 512], F32, tag="mm",
                                                   name=f"qkps{lc}")
                                         for lc in range(QC)]
                                for kc in range(EC):
                                    for lc in range(QC):
                                        nc.tensor.matmul(
                                            psums[lc][:], wt[:, kc, :],
                                            z1T[:, kc, lc * 512:(lc + 1) * 512],
                                            start=(kc == 0), stop=(kc == EC - 1),
                                        )
                                for lc in range(QC):
                                    dst = qk_pair[:, i, lc * 512:(lc + 1) * 512]
                                    if gates["bqk"]:
                                        nc.scalar.activation(
                                            dst, psums[lc][:], AF.Identity,
                                            bias=bqk_sb[:, oc:oc + 1])
                                    else:
                                        nc.any.tensor_copy(out=dst, in_=psums[lc][:])

                            for qc in range(QC):
                                q0 = qc * 512
                                for kt in range(4 * qc, 4 * (qc + 1)):
                                    s0 = kt * P
                                    for par in range(2):
                                        rows = slice(par * D, par * D + D)
                                        pt_buf = PT[par]
                                        pss = ps3s.tile([P, L], F32, tag="st",
                                                        name=f"pss{par}")
                                        if s0 < 512:
                                            segs = [(s0, 512), (512, L)]
                                        else:
                                            segs = [(s0, L)]
                                        lhs = qk_pair[rows, 1, s0:s0 + P]
                                        for (a, b) in segs:
                                            nc.tensor.matmul(pss[:, a:b], lhs,
                                                             qk_pair[rows, 0, a:b],
                                                             start=True, stop=True)
                                        nc.scalar.activation(pt_buf[:, kt, s0:L],
                                                             pss[:, s0:L], AF.Exp)
                                        nc.vector.tensor_tensor(
                                            pt_buf[:, kt, s0:s0 + P],
                                            pt_buf[:, kt, s0:s0 + P],
                                            mask_tri, OP.mult,
                                        )
                                # P@V for both heads: lhsT = [V_h | 1] so psum
                                # row 64 carries the softmax row-sums; the
                                # reciprocal (computed on one row, SBUF — the
                                # custom DVE op reads garbage from PSUM) is
                                # partition-broadcast on the idle GpSimd.
                                for par in range(2):
                                    h = 2 * c + par
                                    pt_buf = PT[par]
                                    pso = ps3v.tile([P, 512], F32, tag="pv")
                                    kts = list(range(4 * (qc + 1)))
                                    for j, kt in enumerate(kts):
                                        st = (j == 0)
                                        sp = (j == len(kts) - 1)
                                        a = max(kt * P, q0)
                                        vsl = v_aug[:, kt, h, :]
                                        rhs = pt_buf[:, kt, a:q0 + 512]
                                        nc.tensor.matmul(pso[0:DA, a - q0:512],
                                                         vsl, rhs,
                                                         start=st, stop=sp)
                                    o_rows = slice(par * D, par * D + D)
                                    srow = recp.tile([P, 512], F32, tag="sr")
                                    nc.vector.tensor_copy(srow[0:1, :],
                                                          pso[D:DA, :])
                                    rec = recp.tile([P, 512], F32, tag="rc")
                                    nc.vector.reciprocal_approx_fast(
                                        rec[0:1, :], srow[0:1, :])
                                    recb = recp.tile([P, 512], F32, tag="rb")
                                    nc.gpsimd.partition_broadcast(
                                        recb[0:D, :], rec[0:1, :])
                                    nc.vector.tensor_tensor(
                                        OT[o_rows, c, q0:q0 + 512],
                                        pso[0:D, :], recb[0:D, :], OP.mult,
                                    )

                    # ------------ ph4: proj + residual (+ LN2 stats) ---------
                    with (
                        tc.tile_pool(name="wpp", bufs=1) as wpp,
                        tc.tile_pool(name="xrp", bufs=3) as xrp,
                        tc.tile_pool(name="x1p", bufs=3) as x1p,
                        tc.tile_pool(name="scr4", bufs=2) as scr4,
                        tc.tile_pool(name="ps4", bufs=4, space="PSUM") as ps4,
                    ):
                        wproj_sb = wpp.tile([P, EC, E], MDT)
                        nc.sync.dma_start(wproj_sb[:], wprojv[:])
                        for t in range(LT):
                            xt = xrp.tile([P, E], F32, tag="xr")
                            nc.sync.dma_start(xt[:], xv[:, t, :])
                            x1t = x1p.tile([P, E], F32, tag="x1")
                            for (c0, cw) in ((0, 512), (512, 256)):
                                pt = ps4.tile([P, 512], F32, tag="mm")
                                for kc in range(EC):
                                    nc.tensor.matmul(
                                        pt[:, :cw], OT[:, kc, t * P:(t + 1) * P],
                                        wproj_sb[:, kc, c0:c0 + cw],
                                        start=(kc == 0), stop=(kc == EC - 1),
                                    )
                                dst = x1t[:, c0:c0 + cw]
                                if gates["bproj"]:
                                    nc.vector.tensor_tensor(
                                        dst, pt[:, :cw],
                                        bproj_sb[:, c0:c0 + cw], OP.add)
                                    nc.vector.tensor_tensor(
                                        dst, dst, xt[:, c0:c0 + cw], OP.add)
                                else:
                                    nc.vector.tensor_tensor(
                                        dst, pt[:, :cw], xt[:, c0:c0 + cw], OP.add)
                            nc.sync.dma_start(x1v[:, t, :], x1t[:])
                            nc.vector.tensor_reduce(m2[:, t:t + 1], x1t[:],
                                                    AX.X, OP.add)
                            sqs = scr4.tile([P, E], F32, tag="sq4")
                            nc.scalar.activation(sqs[:], x1t[:], AF.Square,
                                                 accum_out=sq2[:, t:t + 1])
                        nc.vector.tensor_scalar_mul(m2[:], m2[:], 1.0 / E)
                        nc.vector.tensor_scalar_mul(sq2[:], sq2[:], 1.0 / E)
                        nc.vector.tensor_tensor(tmp8[:], m2[:], m2[:], OP.mult)
                        nc.vector.tensor_tensor(sq2[:], sq2[:], tmp8[:], OP.subtract)
                        nc.scalar.activation(sq2[:], sq2[:], AF.Sqrt, bias=eps_b[:])
                        nc.vector.reciprocal(r2[:], sq2[:])

                # ---------------- ph4.5: LN2 apply + transpose -> z2T -------
                z2T = fmp.tile([P, EC, L], MDT, tag="fm")
                with (
                    tc.tile_pool(name="x1r", bufs=3) as x1r,
                    tc.tile_pool(name="z2p", bufs=3) as z2p,
                    tc.tile_pool(name="ps45", bufs=4, space="PSUM") as ps45,
                ):
                    for t in range(LT):
                        x1t = x1r.tile([P, E], F32, tag="x1r")
                        nc.sync.dma_start(x1t[:], x1v[:, t, :])
                        z2t = z2p.tile([P, E], MDT, tag="z2")
                        nc.vector.tensor_scalar(
                            z2t[:], x1t[:], m2[:, t:t + 1], r2[:, t:t + 1],
                            OP.subtract, OP.mult,
                        )
                        transpose_into(z2T, z2t, t, ps45)

                # ---------------- ph5: fc + selu -> hT ----------------------
                with tc.tile_pool(name="htp", bufs=1) as htp:
                    hT = htp.tile([P, KC2, L], MDT)
                    with tc.tile_pool(name="woa", bufs=1) as woap:
                        # prefetch wout pass-A while the fc matmuls run
                        wo_a = woap.tile([P, KC2, 512], MDT)
                        nc.sync.dma_start(wo_a[:], woutv[:, :, 0:512])
                        with (
                            tc.tile_pool(name="wfcs", bufs=3) as wfs,
                            tc.tile_pool(name="selu", bufs=2) as slp,
                            tc.tile_pool(name="ps5", bufs=4, space="PSUM") as ps5,
                        ):
                            for oc in range(KC2):
                                wt = wfs.tile([P, EC, P], MDT, tag="wfc")
                                nc.sync.dma_start(wt[:],
                                                  wfcv[:, :, oc * P:(oc + 1) * P])
                                for lc in range(QC):
                                    pt = ps5.tile([P, 512], F32, tag="mm")
                                    for kc in range(EC):
                                        nc.tensor.matmul(
                                            pt[:], wt[:, kc, :],
                                            z2T[:, kc, lc * 512:(lc + 1) * 512],
                                            start=(kc == 0), stop=(kc == EC - 1),
                                        )
                                    pe_t = slp.tile([P, 512], F32, tag="pe")
                                    bias = (bfce_sb[:, oc:oc + 1] if gates["bfc"]
                                            else lnla_b[:])
                                    nc.scalar.activation(pe_t[:], pt[:], AF.Exp,
                                                         bias=bias,
                                                         scale=1.0 / SELU_LAMBDA)
                                    a_t = slp.tile([P, 512], F32, tag="at")
                                    nc.vector.tensor_scalar(
                                        a_t[:], pe_t[:], SELU_LA, SELU_LA,
                                        OP.min, OP.subtract)
                                    dst = hT[:, oc, lc * 512:(lc + 1) * 512]
                                    if gates["bfc"]:
                                        rl = slp.tile([P, 512], F32, tag="rl")
                                        nc.vector.tensor_scalar(
                                            rl[:], pt[:], bfcl_sb[:, oc:oc + 1],
                                            0.0, OP.add, OP.max)
                                        nc.vector.tensor_tensor(dst, rl[:], a_t[:],
                                                                OP.add)
                                    else:
                                        nc.vector.scalar_tensor_tensor(
                                            dst, pt[:], 0.0, a_t[:], OP.max, OP.add)

                        # ------------ ph6 pass A: out cols 0:512 ------------
                        with (
                            tc.tile_pool(name="x1sA", bufs=3) as x1s,
                            tc.tile_pool(name="osA", bufs=3) as osp,
                            tc.tile_pool(name="ps6A", bufs=4, space="PSUM") as ps6,
                        ):
                            wo_b = fmp.tile([P, KC2, 256], MDT, tag="fm")
                            nc.sync.dma_start(wo_b[:], woutv[:, :, 512:768])
                            for t in range(LT):
                                pt = ps6.tile([P, 512], F32, tag="mm")
                                for kc in range(KC2):
                                    nc.tensor.matmul(
                                        pt[:], hT[:, kc, t * P:(t + 1) * P],
                                        wo_a[:, kc, :],
                                        start=(kc == 0), stop=(kc == KC2 - 1),
                                    )
                                x1t = x1s.tile([P, 512], F32, tag="x1c")
                                nc.sync.dma_start(x1t[:], x1v[:, t, 0:512])
                                ot = osp.tile([P, 512], F32, tag="ot")
                                if gates["bout"]:
                                    nc.vector.tensor_tensor(
                                        ot[:], pt[:], bout_sb[:, 0:512], OP.add)
                                    nc.vector.tensor_tensor(ot[:], ot[:], x1t[:],
                                                            OP.add)
                                else:
                                    nc.vector.tensor_tensor(ot[:], pt[:], x1t[:],
                                                            OP.add)
                                nc.sync.dma_start(outv[:, t, 0:512], ot[:])

                            # -------- ph6 pass B: out cols 512:768 ----------
                            for t in range(LT):
                                pt = ps6.tile([P, 512], F32, tag="mm")
                                for kc in range(KC2):
                                    nc.tensor.matmul(
                                        pt[:, :256], hT[:, kc, t * P:(t + 1) * P],
                                        wo_b[:, kc, :],
                                        start=(kc == 0), stop=(kc == KC2 - 1),
                                    )
                                x1t = x1s.tile([P, 512], F32, tag="x1c")
                                nc.sync.dma_start(x1t[:, :256], x1v[:, t, 512:768])
                                ot = osp.tile([P, 512], F32, tag="ot")
                                if gates["bout"]:
                                    nc.vector.tensor_tensor(
                                        ot[:, :256], pt[:, :256],
                                        bout_sb[:, 512:768], OP.add)
                                    nc.vector.tensor_tensor(
                                        ot[:, :256], ot[:, :256], x1t[:, :256],
                                        OP.add)
                                else:
                                    nc.vector.tensor_tensor(
                                        ot[:, :256], pt[:, :256], x1t[:, :256],
                                        OP.add)
                                nc.sync.dma_start(outv[:, t, 512:768], ot[:, :256])

    nc.finalize()
    return nc


def kernel(**inputs):
    global _last_results

    mm_dt_name = os.environ.get("KERNEL_MM_DT", "f32r")

    def arr(name):
        return np.ascontiguousarray(np.asarray(inputs[name], dtype=np.float32))

    x = arr("x")                       # [8, 1024, 768]
    g1 = arr("ln1_scale")
    b1 = arr("ln1_bias")
    w_qkv = arr("w_qkv")               # [768, 2304]
    b_qkv = arr("b_qkv")
    w_proj = arr("w_proj")
    b_proj = arr("b_proj")
    g2 = arr("ln2_scale")
    b2 = arr("ln2_bias")
    w_fc = arr("w_fc")
    b_fc = arr("b_fc")
    w_out = arr("w_out")
    b_out = arr("b_out")

    qscale = np.float32(1.0 / np.sqrt(D))

    w3 = w_qkv.reshape(E, H, 3, D)
    qw = (w3[:, :, 0, :].reshape(E, E) * qscale)
    kw = w3[:, :, 1, :].reshape(E, E)
    vw = w3[:, :, 2, :].reshape(E, E)
    wqk = np.ascontiguousarray(
        np.concatenate([qw, kw], axis=1) * g1[:, None]).astype(np.float32)
    wv = np.ascontiguousarray(vw * g1[:, None]).astype(np.float32)

    bq3 = (b1 @ w_qkv + b_qkv).reshape(H, 3, D)
    bqk = np.concatenate(
        [bq3[:, 0, :].reshape(E) * qscale, bq3[:, 1, :].reshape(E)]).astype(np.float32)
    bv = np.ascontiguousarray(bq3[:, 2, :].reshape(E)).astype(np.float32)

    wfc_p = np.ascontiguousarray(
        w_fc * g2[:, None] * np.float32(SELU_LAMBDA)).astype(np.float32)
    bfc_eff = (b2 @ w_fc + b_fc).astype(np.float32)
    bfce = (bfc_eff + np.float32(np.log(SELU_LA))).astype(np.float32)
    bfcl = (bfc_eff * np.float32(SELU_LAMBDA)).astype(np.float32)

    gates = {
        "bqk": bool(np.any(bqk != 0)),
        "bv": bool(np.any(bv != 0)),
        "bproj": bool(np.any(b_proj != 0)),
        "bfc": bool(np.any(bfc_eff != 0)),
        "bout": bool(np.any(b_out != 0)),
    }

    key = (tuple(sorted(gates.items())), mm_dt_name)
    if key not in _build_cache:
        _build_cache[key] = _build(gates, mm_dt_name)
    nc = _build_cache[key]

    wdt = np.float32 if mm_dt_name == "f32r" else ml_dtypes.bfloat16

    def wcast(a):
        return np.ascontiguousarray(a.astype(wdt))

    base = {
        "wqk": wcast(wqk), "wv": wcast(wv),
        "wproj": wcast(w_proj),
        "wfc": wcast(wfc_p),
        "wout": wcast(w_out),
    }
    if gates["bqk"]:
        base["bqk"] = bqk
    if gates["bv"]:
        base["bv"] = bv
    if gates["bproj"]:
        base["bproj"] = np.ascontiguousarray(b_proj)
    if gates["bfc"]:
        base["bfce"] = bfce
        base["bfcl"] = bfcl
    if gates["bout"]:
        base["bout"] = np.ascontiguousarray(b_out)

    in_maps = [dict(base, x=np.ascontiguousarray(x[c])) for c in range(NCORES)]
    res = bass_utils.run_bass_kernel_spmd(nc, in_maps, core_ids=list(range(NCORES)))
    _last_results = res
    out = np.stack([res.results[c]["out"] for c in range(NCORES)], axis=0)
    return out.astype(np.float32)
